# revision 4
# baseline (speedup 1.0000x reference)
"""Trainium2 Bass kernel for nn_Attention_15771119911478 (RBF attention w/ RoPE).

Sharding: core h (of 8) computes head h for both batches. Per-core output is
the head's contribution to out @ Wo.T, transposed ([2*64(e), 2048(s)]), minus
a per-row factor exp(-g*qn[s]) which is applied on the host (it commutes
through the Wo projection). Host sums the 8 per-core partials.

Device math per core (batch b packed on partition halves):
  qro = (A_q q^T) * C + (B_q q^T) * S          (RoPE as two projections)
  kro = 2g * [(A_k q^T) * C + (B_k q^T) * S]   (2g folded into Wk)
  scT[t,s] = exp(kro[:,t].qro[:,s] - g*kn[t])  (kn bias folded into ACT exp)
  out_hT = vh^T @ (scT masked t<=s)
  partial_T = Wo_h @ out_hT                     -> DMA out [128, 2048] f32
"""
import os
import sys

sys.path.insert(0, "/opt/trn_rl_repo")

import numpy as np
import ml_dtypes

S = 2048
D = 64
H = 8
B = 2
N_CORES = 8
SCALE = 1.0 / 8.0  # 1/sqrt(64)
BF16 = ml_dtypes.bfloat16

_PROG = None  # cached (nc, tensors) after first build
LAST_RESULTS = None  # BassKernelResults of last run (for test.py)


def _build_program():
    import concourse.bass as bass
    import concourse.bacc as bacc
    import concourse.tile as tile
    from concourse import mybir

    f32 = mybir.dt.float32
    bf16 = mybir.dt.bfloat16
    Exp = mybir.ActivationFunctionType.Exp

    nc = bacc.Bacc(
        "TRN2",
        target_bir_lowering=False,
        debug=False,
        enable_asserts=False,
        num_devices=N_CORES,
    )

    def din(name, shape, dt):
        return nc.dram_tensor(name, shape, dt, kind="ExternalInput").ap()

    t_qT = din("qT", [128, S], bf16)
    t_wqa = din("wqa", [128, 64], bf16)
    t_wqb = din("wqb", [128, 64], bf16)
    t_wka = din("wka", [128, 64], bf16)
    t_wkb = din("wkb", [128, 64], bf16)
    t_wv = din("wv", [128, 64], bf16)
    t_wo = din("wo", [128, 64], bf16)
    t_cos = din("cosb", [128, S], bf16)
    t_sin = din("sinb", [128, S], bf16)
    t_kn0 = din("knb0", [128, 16], f32)
    t_kn1 = din("knb1", [128, 16], f32)
    t_mask = din("mask", [128, 128], bf16)
    t_out = nc.dram_tensor("out", [128, S], f32, kind="ExternalOutput").ap()

    with tile.TileContext(nc) as tc:
        with (
            tc.tile_pool(name="const", bufs=1) as const,
            tc.tile_pool(name="big", bufs=1) as big,
            tc.tile_pool(name="scp", bufs=1) as scp,
        ):
            # ---- load inputs to SBUF ----
            qT = big.tile([128, S], bf16, tag="qT")
            nc.sync.dma_start(qT[:], t_qT[:])
            wqa = const.tile([128, 64], bf16, tag="wqa")
            nc.sync.dma_start(wqa[:], t_wqa[:])
            wqb = const.tile([128, 64], bf16, tag="wqb")
            nc.sync.dma_start(wqb[:], t_wqb[:])
            wka = const.tile([128, 64], bf16, tag="wka")
            nc.sync.dma_start(wka[:], t_wka[:])
            wkb = const.tile([128, 64], bf16, tag="wkb")
            nc.sync.dma_start(wkb[:], t_wkb[:])
            wv = const.tile([128, 64], bf16, tag="wv")
            nc.sync.dma_start(wv[:], t_wv[:])
            wo = const.tile([128, 64], bf16, tag="wo")
            nc.sync.dma_start(wo[:], t_wo[:])
            kn = [
                const.tile([128, 16], f32, tag="kn0", name="kn0"),
                const.tile([128, 16], f32, tag="kn1", name="kn1"),
            ]
            nc.sync.dma_start(kn[0][:], t_kn0[:])
            nc.sync.dma_start(kn[1][:], t_kn1[:])
            mask = const.tile([128, 128], bf16, tag="mask")
            nc.sync.dma_start(mask[:], t_mask[:])
            cosb = big.tile([128, S], bf16, tag="cosb")
            nc.sync.dma_start(cosb[:], t_cos[:])
            sinb = big.tile([128, S], bf16, tag="sinb")
            nc.sync.dma_start(sinb[:], t_sin[:])

            # preload ACT exp table early (overlaps with projections)
            scratch = const.tile([128, 1], f32, tag="scratch")
            nc.scalar.activation(scratch[:], kn[0][:, 0:1], Exp)

            # ---- q/k projections + RoPE (64x64 tiles T0/T10) ----
            qro = big.tile([128, S], bf16, tag="qro")
            kro = big.tile([128, S], bf16, tag="kro")
            with tc.tile_pool(name="projp", bufs=1, space="PSUM") as projp:
                for wa, wb, dst in ((wqa, wqb, qro), (wka, wkb, kro)):
                    pa = projp.tile([128, S], f32, tag="projA")
                    pb = projp.tile([128, S], f32, tag="projB")
                    for c in range(4):
                        sl = slice(c * 512, (c + 1) * 512)
                        for w, p in ((wa, pa), (wb, pb)):
                            nc.tensor.matmul(
                                p[0:64, sl], w[0:64, :], qT[0:64, sl],
                                start=True, stop=True, tile_position=(0, 0),
                            )
                            nc.tensor.matmul(
                                p[64:128, sl], w[64:128, :], qT[64:128, sl],
                                start=True, stop=True, tile_position=(64, 64),
                            )
                    tmp1 = big.tile([128, S], bf16, tag="ropetmp1")
                    tmp2 = big.tile([128, S], bf16, tag="ropetmp2")
                    nc.vector.tensor_mul(tmp1[:], pa[:], cosb[:])
                    nc.vector.tensor_mul(tmp2[:], pb[:], sinb[:])
                    nc.vector.tensor_add(dst[:], tmp1[:], tmp2[:])

            # ---- v projection (64x128 tiles T0/T8) ----
            vsb = [
                big.tile([128, 1024], bf16, tag="vsb0", name="vsb0"),
                big.tile([128, 1024], bf16, tag="vsb1", name="vsb1"),
            ]
            with tc.tile_pool(name="vp", bufs=1, space="PSUM") as vpool:
                vp = [
                    vpool.tile([128, 1024], f32, tag="vp0", name="vp0"),
                    vpool.tile([128, 1024], f32, tag="vp1", name="vp1"),
                ]
                for j in range(16):
                    js = slice(j * 128, (j + 1) * 128)
                    ds = slice(j * 64, (j + 1) * 64)
                    nc.tensor.matmul(
                        vp[0][:, ds], qT[0:64, js], wv[0:64, :],
                        start=True, stop=True, tile_position=(0, 0),
                    )
                    nc.tensor.matmul(
                        vp[1][:, ds], qT[64:128, js], wv[64:128, :],
                        start=True, stop=True, tile_position=(64, 0),
                    )
                nc.vector.tensor_copy(vsb[0][:], vp[0][:])
                nc.vector.tensor_copy(vsb[1][:], vp[1][:])

            # ---- attention ----
            # sc strips: scs[(j,b)] covers scores^T rows t in [128j,128j+128),
            # cols s in [128j, 2048)
            scs = {}
            for j in range(16):
                cols = S - j * 128
                for b in (0, 1):
                    scs[(j, b)] = scp.tile(
                        [128, cols], bf16, tag=f"sc_{j}_{b}", name=f"sc_{j}_{b}"
                    )
            osb = big.tile([128, S], bf16, tag="osb")
            # attention runs in two s-halves (sig): qk+exp for strips
            # intersecting the half, then sv accumulation into pout_sig.
            # pout_sig layout: b0 -> [0:64, 0:1024] (banks 0-1),
            # b1 -> [64:128, 1024:2048] (banks 2-3): batch-disjoint banks so
            # the two accumulation groups never share a psum zero region.
            for sig in (0, 1):
                s_lo, s_hi = sig * 1024, (sig + 1) * 1024
                nstrips = 8 * (sig + 1)
                with (
                    tc.tile_pool(name=f"qkp{sig}", bufs=2, space="PSUM") as qkp,
                    tc.tile_pool(name=f"op{sig}", bufs=1, space="PSUM") as opool,
                ):
                    pout = opool.tile(
                        [128, 2048], f32, tag=f"pout{sig}", name=f"pout{sig}"
                    )
                    # qk matmuls + exp (64x128 tiles T0/T8)
                    for j in range(nstrips):
                        base = j * 128
                        a0 = max(s_lo, base)
                        w = s_hi - a0
                        for b in (0, 1):
                            rows = slice(0, 64) if b == 0 else slice(64, 128)
                            tp = (0, 0) if b == 0 else (64, 0)
                            qk = qkp.tile([128, 1024], f32, tag="qk")
                            for off in range(0, w, 512):
                                wc = min(512, w - off)
                                ssl = slice(a0 + off, a0 + off + wc)
                                nc.tensor.matmul(
                                    qk[:, off : off + wc],
                                    kro[rows, base : base + 128],
                                    qro[rows, ssl],
                                    start=True, stop=True, tile_position=tp,
                                )
                            nc.scalar.activation(
                                scs[(j, b)][:, a0 - base : a0 - base + w],
                                qk[:, 0:w],
                                Exp,
                                bias=kn[b][:, j : j + 1],
                                scale=1.0,
                            )
                        # mask the diagonal block (strictly-lower part -> 0)
                        if a0 == base:
                            for b in (0, 1):
                                nc.vector.tensor_mul(
                                    scs[(j, b)][:, 0:128],
                                    scs[(j, b)][:, 0:128],
                                    mask[:],
                                )
                    # sv matmuls (128x64 col tiles T0/T1) into pout
                    for ci in (0, 1):
                        c = 2 * sig + ci
                        lo, hi = c * 512, (c + 1) * 512
                        nj = min(4 * (c + 1), 16)
                        for j in range(nj):
                            a = max(lo, j * 128)
                            for b in (0, 1):
                                orows = slice(0, 64) if b == 0 else slice(64, 128)
                                tp = (0, 0) if b == 0 else (0, 64)
                                po = 1024 * b + a - s_lo
                                nc.tensor.matmul(
                                    pout[orows, po : po + hi - a],
                                    vsb[b][:, j * 64 : (j + 1) * 64],
                                    scs[(j, b)][:, a - j * 128 : hi - j * 128],
                                    start=(j == 0),
                                    stop=(j == nj - 1),
                                    tile_position=tp,
                                )
                    nc.vector.tensor_copy(
                        osb[0:64, s_lo:s_hi], pout[0:64, 0:1024]
                    )
                    nc.vector.tensor_copy(
                        osb[64:128, s_lo:s_hi], pout[64:128, 1024:2048]
                    )

            # ---- Wo projection (64x64 tiles T0/T10) ----
            outsb = big.tile([128, S], f32, tag="outsb")
            with tc.tile_pool(name="fin", bufs=1, space="PSUM") as finp:
                pfin = finp.tile([128, S], f32, tag="pfin")
                for c in range(4):
                    sl = slice(c * 512, (c + 1) * 512)
                    nc.tensor.matmul(
                        pfin[0:64, sl], wo[0:64, :], osb[0:64, sl],
                        start=True, stop=True, tile_position=(0, 0),
                    )
                    nc.tensor.matmul(
                        pfin[64:128, sl], wo[64:128, :], osb[64:128, sl],
                        start=True, stop=True, tile_position=(64, 64),
                    )
                nc.vector.tensor_copy(outsb[:], pfin[:])
            nc.sync.dma_start(t_out[:], outsb[:])

    nc.compile()
    return nc


def _get_program():
    global _PROG
    if _PROG is None:
        _PROG = _build_program()
    return _PROG


def _prep_inputs(q, Wq, Wk, Wv, Wo, gamma):
    """Build the per-core in_maps (all host-side numpy)."""
    q = np.asarray(q, np.float32)
    Wq = np.asarray(Wq, np.float32)
    Wk = np.asarray(Wk, np.float32)
    Wv = np.asarray(Wv, np.float32)
    Wo = np.asarray(Wo, np.float32)
    gamma = np.asarray(gamma, np.float32)

    perm = np.concatenate([np.arange(0, 64, 2), np.arange(1, 64, 2)])
    f = np.arange(32, dtype=np.float64)
    freqs = 1.0 / (10000.0 ** (2 * f / 64))
    ang = np.arange(S, dtype=np.float64)[:, None] * freqs[None, :]
    cosr = np.cos(ang).T.astype(np.float32)  # [32, S]
    sinr = np.sin(ang).T.astype(np.float32)
    C64 = np.concatenate([cosr, cosr], 0)  # [64, S]
    S64 = np.concatenate([sinr, sinr], 0)
    cosb = np.concatenate([C64, C64], 0).astype(BF16)  # [128, S]
    sinb = np.concatenate([S64, S64], 0).astype(BF16)

    qT_packed = np.concatenate([q[0].T, q[1].T], 0).astype(BF16)  # [128, S]
    mask = np.triu(np.ones((128, 128), np.float32)).astype(BF16)

    def dup(x):
        return np.concatenate([x, x], 0)

    in_maps = []
    qn_exp = np.zeros((B, H, S), np.float32)
    for h in range(H):
        g = float(gamma[h]) * SCALE
        Wq_h = Wq[h * 64 : (h + 1) * 64]
        Wk_h = Wk[h * 64 : (h + 1) * 64]
        Wv_h = Wv[h * 64 : (h + 1) * 64]
        Wo_h = Wo[:, h * 64 : (h + 1) * 64]  # [64(e), 64(d)]
        A_q = Wq_h[perm]
        B_q = np.concatenate([-Wq_h[1::2], Wq_h[0::2]], 0)
        A_k = Wk_h[perm] * (2.0 * g)
        B_k = np.concatenate([-Wk_h[1::2], Wk_h[0::2]], 0) * (2.0 * g)

        knb = []
        for b in range(B):
            kh = q[b] @ Wk_h.T
            kn = (kh * kh).sum(-1)  # [S]
            knb.append((-g * kn).reshape(16, 128).T.astype(np.float32))
            qh = q[b] @ Wq_h.T
            qn = (qh * qh).sum(-1)
            qn_exp[b, h] = np.exp(-g * qn)

        in_maps.append(
            {
                "qT": qT_packed,
                "wqa": dup(A_q.T).astype(BF16),
                "wqb": dup(B_q.T).astype(BF16),
                "wka": dup(A_k.T).astype(BF16),
                "wkb": dup(B_k.T).astype(BF16),
                "wv": dup(Wv_h.T).astype(BF16),
                "wo": dup(Wo_h.T).astype(BF16),
                "cosb": cosb,
                "sinb": sinb,
                "knb0": np.ascontiguousarray(knb[0]),
                "knb1": np.ascontiguousarray(knb[1]),
                "mask": mask,
            }
        )
    return in_maps, qn_exp


def kernel(q, Wq, Wk, Wv, Wo, gamma):
    global LAST_RESULTS
    from concourse import bass_utils

    nc = _get_program()
    in_maps, qn_exp = _prep_inputs(q, Wq, Wk, Wv, Wo, gamma)
    trace = bool(int(os.environ.get("KERNEL_TRACE", "0")))
    res = bass_utils.run_bass_kernel_spmd(
        nc, in_maps, core_ids=list(range(N_CORES)), trace=trace
    )
    LAST_RESULTS = res

    final = np.zeros((B, S, D), np.float32)
    for h in range(H):
        o = np.asarray(res.results[h]["out"], np.float32)  # [128, S]
        for b in range(B):
            final[b] += (o[b * 64 : (b + 1) * 64, :] * qn_exp[b, h][None, :]).T
    return final


# revision 5
# speedup vs baseline: 1.0126x; 1.0126x over previous
"""Trainium2 Bass kernel for nn_Attention_15771119911478 (RBF attention w/ RoPE).

Sharding: core h (of 8) computes head h for both batches. Per-core output is
the head's contribution to out @ Wo.T, transposed ([2*64(e), 2048(s)]), minus
a per-row factor exp(-g*qn[s]) which is applied on the host (it commutes
through the Wo projection). Host sums the 8 per-core partials.

Device math per core (batch b packed on partition halves):
  qro = (A_q q^T) * C + (B_q q^T) * S          (RoPE as two projections)
  kro = 2g * [(A_k q^T) * C + (B_k q^T) * S]   (2g folded into Wk)
  scT[t,s] = exp(kro[:,t].qro[:,s] - g*kn[t])  (kn bias folded into ACT exp)
  out_hT = vh^T @ (scT masked t<=s)
  partial_T = Wo_h @ out_hT                     -> DMA out [128, 2048] f32
"""
import os
import sys

sys.path.insert(0, "/opt/trn_rl_repo")

import numpy as np
import ml_dtypes

S = 2048
D = 64
H = 8
B = 2
N_CORES = 8
SCALE = 1.0 / 8.0  # 1/sqrt(64)
BF16 = ml_dtypes.bfloat16

_PROG = None  # cached (nc, tensors) after first build
LAST_RESULTS = None  # BassKernelResults of last run (for test.py)


def _build_program():
    import concourse.bass as bass
    import concourse.bacc as bacc
    import concourse.tile as tile
    from concourse import mybir

    f32 = mybir.dt.float32
    bf16 = mybir.dt.bfloat16
    Exp = mybir.ActivationFunctionType.Exp

    nc = bacc.Bacc(
        "TRN2",
        target_bir_lowering=False,
        debug=False,
        enable_asserts=False,
        num_devices=N_CORES,
    )

    def din(name, shape, dt):
        return nc.dram_tensor(name, shape, dt, kind="ExternalInput").ap()

    t_qT = din("qT", [128, S], bf16)
    t_wqa = din("wqa", [128, 64], bf16)
    t_wqb = din("wqb", [128, 64], bf16)
    t_wka = din("wka", [128, 64], bf16)
    t_wkb = din("wkb", [128, 64], bf16)
    t_wv = din("wv", [128, 64], bf16)
    t_cos = din("cosb", [128, S], bf16)
    t_sin = din("sinb", [128, S], bf16)
    t_kn0 = din("knb0", [128, 16], f32)
    t_kn1 = din("knb1", [128, 16], f32)
    t_mask = din("mask", [128, 128], bf16)
    t_out = nc.dram_tensor("out", [128, S], f32, kind="ExternalOutput").ap()

    with tile.TileContext(nc) as tc:
        with (
            tc.tile_pool(name="const", bufs=1) as const,
            tc.tile_pool(name="big", bufs=1) as big,
            tc.tile_pool(name="scp", bufs=1) as scp,
        ):
            # ---- load inputs to SBUF ----
            qT = big.tile([128, S], bf16, tag="qT")
            cosb = big.tile([128, S], bf16, tag="cosb")
            sinb = big.tile([128, S], bf16, tag="sinb")
            wqa = const.tile([128, 64], bf16, tag="wqa")
            nc.sync.dma_start(wqa[:], t_wqa[:])
            wqb = const.tile([128, 64], bf16, tag="wqb")
            nc.sync.dma_start(wqb[:], t_wqb[:])
            wka = const.tile([128, 64], bf16, tag="wka")
            nc.sync.dma_start(wka[:], t_wka[:])
            wkb = const.tile([128, 64], bf16, tag="wkb")
            nc.sync.dma_start(wkb[:], t_wkb[:])
            wv = const.tile([128, 64], bf16, tag="wv")
            nc.sync.dma_start(wv[:], t_wv[:])
            kn = [
                const.tile([128, 16], f32, tag="kn0", name="kn0"),
                const.tile([128, 16], f32, tag="kn1", name="kn1"),
            ]
            nc.sync.dma_start(kn[0][:], t_kn0[:])
            nc.sync.dma_start(kn[1][:], t_kn1[:])
            mask = const.tile([128, 128], bf16, tag="mask")
            nc.sync.dma_start(mask[:], t_mask[:])
            # chunked loads so proj/rope of chunk c can start right away
            for c in range(4):
                sl = slice(c * 512, (c + 1) * 512)
                nc.sync.dma_start(qT[:, sl], t_qT[:, sl])
                nc.sync.dma_start(cosb[:, sl], t_cos[:, sl])
                nc.sync.dma_start(sinb[:, sl], t_sin[:, sl])

            # preload ACT exp table early (overlaps with projections)
            scratch = const.tile([128, 1], f32, tag="scratch")
            nc.scalar.activation(scratch[:], kn[0][:, 0:1], Exp)

            # ---- q/k projections + RoPE (64x64 tiles T0/T10) ----
            qro = big.tile([128, S], bf16, tag="qro")
            kro = big.tile([128, S], bf16, tag="kro")
            with tc.tile_pool(name="projp", bufs=2, space="PSUM") as projp:
                for c in range(4):
                    sl = slice(c * 512, (c + 1) * 512)
                    for wa, wb, dst in ((wqa, wqb, qro), (wka, wkb, kro)):
                        pa = projp.tile([128, 512], f32, tag="projA")
                        pb = projp.tile([128, 512], f32, tag="projB")
                        for w, p in ((wa, pa), (wb, pb)):
                            nc.tensor.matmul(
                                p[0:64, :], w[0:64, :], qT[0:64, sl],
                                start=True, stop=True, tile_position=(0, 0),
                            )
                            nc.tensor.matmul(
                                p[64:128, :], w[64:128, :], qT[64:128, sl],
                                start=True, stop=True, tile_position=(64, 64),
                            )
                        tmp1 = big.tile([128, 512], bf16, tag="ropetmp1")
                        tmp2 = big.tile([128, 512], bf16, tag="ropetmp2")
                        nc.vector.tensor_mul(tmp1[:], pa[:], cosb[:, sl])
                        nc.vector.tensor_mul(tmp2[:], pb[:], sinb[:, sl])
                        nc.vector.tensor_add(dst[:, sl], tmp1[:], tmp2[:])

            # ---- v projection (64x128 tiles T0/T8) ----
            vsb = [
                big.tile([128, 1024], bf16, tag="vsb0", name="vsb0"),
                big.tile([128, 1024], bf16, tag="vsb1", name="vsb1"),
            ]
            with tc.tile_pool(name="vp", bufs=1, space="PSUM") as vpool:
                vp = [
                    vpool.tile([128, 1024], f32, tag="vp0", name="vp0"),
                    vpool.tile([128, 1024], f32, tag="vp1", name="vp1"),
                ]
                for j in range(16):
                    js = slice(j * 128, (j + 1) * 128)
                    ds = slice(j * 64, (j + 1) * 64)
                    nc.tensor.matmul(
                        vp[0][:, ds], qT[0:64, js], wv[0:64, :],
                        start=True, stop=True, tile_position=(0, 0),
                    )
                    nc.tensor.matmul(
                        vp[1][:, ds], qT[64:128, js], wv[64:128, :],
                        start=True, stop=True, tile_position=(64, 0),
                    )
                nc.vector.tensor_copy(vsb[0][:], vp[0][:])
                nc.vector.tensor_copy(vsb[1][:], vp[1][:])

            # ---- attention ----
            # sc strips: scs[(j,b)] covers scores^T rows t in [128j,128j+128),
            # cols s in [128j, 2048)
            scs = {}
            for j in range(16):
                cols = S - j * 128
                for b in (0, 1):
                    scs[(j, b)] = scp.tile(
                        [128, cols], bf16, tag=f"sc_{j}_{b}", name=f"sc_{j}_{b}"
                    )
            outsb = big.tile([128, S], f32, tag="outsb")
            # attention runs in two s-halves (sig): qk+exp for strips
            # intersecting the half, then sv accumulation into pout_sig.
            # pout_sig layout: b0 -> [0:64, 0:1024] (banks 0-1),
            # b1 -> [64:128, 1024:2048] (banks 2-3): batch-disjoint banks so
            # the two accumulation groups never share a psum zero region.
            for sig in (0, 1):
                s_lo, s_hi = sig * 1024, (sig + 1) * 1024
                nstrips = 8 * (sig + 1)
                with (
                    tc.tile_pool(name=f"qkp{sig}", bufs=2, space="PSUM") as qkp,
                    tc.tile_pool(name=f"op{sig}", bufs=1, space="PSUM") as opool,
                ):
                    pout = opool.tile(
                        [128, 2048], f32, tag=f"pout{sig}", name=f"pout{sig}"
                    )
                    # qk matmuls + exp (64x128 tiles T0/T8)
                    for j in range(nstrips):
                        base = j * 128
                        a0 = max(s_lo, base)
                        w = s_hi - a0
                        for b in (0, 1):
                            rows = slice(0, 64) if b == 0 else slice(64, 128)
                            tp = (0, 0) if b == 0 else (64, 0)
                            qk = qkp.tile([128, 1024], f32, tag="qk")
                            for off in range(0, w, 512):
                                wc = min(512, w - off)
                                ssl = slice(a0 + off, a0 + off + wc)
                                nc.tensor.matmul(
                                    qk[:, off : off + wc],
                                    kro[rows, base : base + 128],
                                    qro[rows, ssl],
                                    start=True, stop=True, tile_position=tp,
                                )
                            nc.scalar.activation(
                                scs[(j, b)][:, a0 - base : a0 - base + w],
                                qk[:, 0:w],
                                Exp,
                                bias=kn[b][:, j : j + 1],
                                scale=1.0,
                            )
                        # mask the diagonal block (strictly-lower part -> 0)
                        if a0 == base:
                            for b in (0, 1):
                                nc.vector.tensor_mul(
                                    scs[(j, b)][:, 0:128],
                                    scs[(j, b)][:, 0:128],
                                    mask[:],
                                )
                    # sv matmuls (128x64 col tiles T0/T1) into pout
                    for ci in (0, 1):
                        c = 2 * sig + ci
                        lo, hi = c * 512, (c + 1) * 512
                        nj = min(4 * (c + 1), 16)
                        for j in range(nj):
                            a = max(lo, j * 128)
                            for b in (0, 1):
                                orows = slice(0, 64) if b == 0 else slice(64, 128)
                                tp = (0, 0) if b == 0 else (0, 64)
                                po = 1024 * b + a - s_lo
                                nc.tensor.matmul(
                                    pout[orows, po : po + hi - a],
                                    vsb[b][:, j * 64 : (j + 1) * 64],
                                    scs[(j, b)][:, a - j * 128 : hi - j * 128],
                                    start=(j == 0),
                                    stop=(j == nj - 1),
                                    tile_position=tp,
                                )
                    nc.vector.tensor_copy(
                        outsb[0:64, s_lo:s_hi], pout[0:64, 0:1024]
                    )
                    nc.vector.tensor_copy(
                        outsb[64:128, s_lo:s_hi], pout[64:128, 1024:2048]
                    )
                nc.sync.dma_start(t_out[:, s_lo:s_hi], outsb[:, s_lo:s_hi])

    nc.compile()
    return nc


def _get_program():
    global _PROG
    if _PROG is None:
        _PROG = _build_program()
    return _PROG


def _prep_inputs(q, Wq, Wk, Wv, Wo, gamma):
    """Build the per-core in_maps (all host-side numpy)."""
    q = np.asarray(q, np.float32)
    Wq = np.asarray(Wq, np.float32)
    Wk = np.asarray(Wk, np.float32)
    Wv = np.asarray(Wv, np.float32)
    Wo = np.asarray(Wo, np.float32)
    gamma = np.asarray(gamma, np.float32)

    perm = np.concatenate([np.arange(0, 64, 2), np.arange(1, 64, 2)])
    f = np.arange(32, dtype=np.float64)
    freqs = 1.0 / (10000.0 ** (2 * f / 64))
    ang = np.arange(S, dtype=np.float64)[:, None] * freqs[None, :]
    cosr = np.cos(ang).T.astype(np.float32)  # [32, S]
    sinr = np.sin(ang).T.astype(np.float32)
    C64 = np.concatenate([cosr, cosr], 0)  # [64, S]
    S64 = np.concatenate([sinr, sinr], 0)
    cosb = np.concatenate([C64, C64], 0).astype(BF16)  # [128, S]
    sinb = np.concatenate([S64, S64], 0).astype(BF16)

    qT_packed = np.concatenate([q[0].T, q[1].T], 0).astype(BF16)  # [128, S]
    mask = np.triu(np.ones((128, 128), np.float32)).astype(BF16)

    def dup(x):
        return np.concatenate([x, x], 0)

    in_maps = []
    qn_exp = np.zeros((B, H, S), np.float32)
    for h in range(H):
        g = float(gamma[h]) * SCALE
        Wq_h = Wq[h * 64 : (h + 1) * 64]
        Wk_h = Wk[h * 64 : (h + 1) * 64]
        Wv_h = Wv[h * 64 : (h + 1) * 64]
        Wo_h = Wo[:, h * 64 : (h + 1) * 64]  # [64(e), 64(d)]
        W_vo = Wv_h.T @ Wo_h.T  # [64(i), 64(e)] : q @ W_vo = vh @ Wo_h.T
        A_q = Wq_h[perm]
        B_q = np.concatenate([-Wq_h[1::2], Wq_h[0::2]], 0)
        A_k = Wk_h[perm] * (2.0 * g)
        B_k = np.concatenate([-Wk_h[1::2], Wk_h[0::2]], 0) * (2.0 * g)

        knb = []
        for b in range(B):
            kh = q[b] @ Wk_h.T
            kn = (kh * kh).sum(-1)  # [S]
            knb.append((-g * kn).reshape(16, 128).T.astype(np.float32))
            qh = q[b] @ Wq_h.T
            qn = (qh * qh).sum(-1)
            qn_exp[b, h] = np.exp(-g * qn)

        in_maps.append(
            {
                "qT": qT_packed,
                "wqa": dup(A_q.T).astype(BF16),
                "wqb": dup(B_q.T).astype(BF16),
                "wka": dup(A_k.T).astype(BF16),
                "wkb": dup(B_k.T).astype(BF16),
                "wv": dup(W_vo).astype(BF16),
                "cosb": cosb,
                "sinb": sinb,
                "knb0": np.ascontiguousarray(knb[0]),
                "knb1": np.ascontiguousarray(knb[1]),
                "mask": mask,
            }
        )
    return in_maps, qn_exp


def kernel(q, Wq, Wk, Wv, Wo, gamma):
    global LAST_RESULTS
    from concourse import bass_utils

    nc = _get_program()
    in_maps, qn_exp = _prep_inputs(q, Wq, Wk, Wv, Wo, gamma)
    trace = bool(int(os.environ.get("KERNEL_TRACE", "0")))
    res = bass_utils.run_bass_kernel_spmd(
        nc, in_maps, core_ids=list(range(N_CORES)), trace=trace
    )
    LAST_RESULTS = res

    final = np.zeros((B, S, D), np.float32)
    for h in range(H):
        o = np.asarray(res.results[h]["out"], np.float32)  # [128, S]
        for b in range(B):
            final[b] += (o[b * 64 : (b + 1) * 64, :] * qn_exp[b, h][None, :]).T
    return final


# revision 12
# speedup vs baseline: 1.1138x; 1.1000x over previous
"""Trainium2 Bass kernel for nn_Attention_15771119911478 (RBF attention w/ RoPE).

Sharding: core h (of 8) computes head h for both batches. Per-core output is
the head's contribution to out @ Wo.T, transposed ([2*64(e), 2048(s)]), minus
a per-row factor exp(-g*qn[s]) which is applied on the host (it commutes
through the Wo projection). Host sums the 8 per-core partials.

Device math per core (batch b packed on partition halves):
  qro = (A_q q^T) * C + (B_q q^T) * S          (RoPE as two projections)
  kro = 2g * [(A_k q^T) * C + (B_k q^T) * S]   (2g folded into Wk)
  scT[t,s] = exp(kro[:,t].qro[:,s] - g*kn[t])  (kn bias folded into ACT exp)
  out_hT = vh^T @ (scT masked t<=s)
  partial_T = Wo_h @ out_hT                     -> DMA out [128, 2048] f32
"""
import os
import sys

sys.path.insert(0, "/opt/trn_rl_repo")

import numpy as np
import ml_dtypes

S = 2048
D = 64
H = 8
B = 2
N_CORES = 8
SCALE = 1.0 / 8.0  # 1/sqrt(64)
BF16 = ml_dtypes.bfloat16

_PROG = None  # cached (nc, tensors) after first build
LAST_RESULTS = None  # BassKernelResults of last run (for test.py)


def _build_program():
    import concourse.bass as bass
    import concourse.bacc as bacc
    import concourse.tile as tile
    from concourse import mybir

    f32 = mybir.dt.float32
    bf16 = mybir.dt.bfloat16
    Exp = mybir.ActivationFunctionType.Exp

    nc = bacc.Bacc(
        "TRN2",
        target_bir_lowering=False,
        debug=False,
        enable_asserts=False,
        num_devices=N_CORES,
    )

    def din(name, shape, dt):
        return nc.dram_tensor(name, shape, dt, kind="ExternalInput").ap()

    t_qT = din("qT", [128, S], bf16)
    t_wcat = din("wcat", [128, 448], bf16)  # wqa|wqb|wka|wkb|wv|mask
    t_cos = din("cosb", [128, S], bf16)
    t_sin = din("sinb", [128, S], bf16)
    t_kncat = din("kncat", [128, 32], f32)  # knb0|knb1
    t_out = nc.dram_tensor("out", [128, S], f32, kind="ExternalOutput").ap()

    with tile.TileContext(nc) as tc:
        with (
            tc.tile_pool(name="const", bufs=1) as const,
            tc.tile_pool(name="big", bufs=1) as big,
            tc.tile_pool(name="scp", bufs=1) as scp,
        ):
            # ---- load inputs to SBUF ----
            qT = big.tile([128, S], bf16, tag="qT")
            cosb = big.tile([128, S], bf16, tag="cosb")
            sinb = big.tile([128, S], bf16, tag="sinb")
            wcat = const.tile([128, 448], bf16, tag="wcat")
            kncat = const.tile([128, 32], f32, tag="kncat")
            nc.sync.dma_start(wcat[:], t_wcat[:])
            wqa, wqb = wcat[:, 0:64], wcat[:, 64:128]
            wka, wkb = wcat[:, 128:192], wcat[:, 192:256]
            wv = wcat[:, 256:320]
            mask = wcat[:, 320:448]
            # chunked loads, first chunk right after the consts so the
            # proj/rope/attention pipeline starts immediately
            for c in range(4):
                sl = slice(c * 512, (c + 1) * 512)
                nc.sync.dma_start(qT[:, sl], t_qT[:, sl])
                nc.sync.dma_start(cosb[:, sl], t_cos[:, sl])
                nc.sync.dma_start(sinb[:, sl], t_sin[:, sl])
                if c == 0:
                    nc.sync.dma_start(kncat[:], t_kncat[:])
            kn = [kncat[:, 0:16], kncat[:, 16:32]]

            # preload ACT exp table early (overlaps with projections)
            scratch = const.tile([128, 1], f32, tag="scratch")
            nc.scalar.activation(scratch[:], kncat[:, 0:1], Exp)

            # ---- q/k projections + RoPE (64x64 tiles T0/T10) ----
            qro = big.tile([128, S], bf16, tag="qro")
            kro = big.tile([128, S], bf16, tag="kro")
            vsb = [
                big.tile([128, 1024], bf16, tag="vsb0", name="vsb0"),
                big.tile([128, 1024], bf16, tag="vsb1", name="vsb1"),
            ]
            # sc strips: scs[(j,b)] covers scores^T rows t in [128j,128j+128),
            # cols s in [128j, 2048)
            scs = {}
            for j in range(16):
                cols = S - j * 128
                for b in (0, 1):
                    scs[(j, b)] = scp.tile(
                        [128, cols], bf16, tag=f"sc_{j}_{b}", name=f"sc_{j}_{b}"
                    )
            outsb = big.tile([128, S], f32, tag="outsb")

            def proj_chunks(projp, cs):
                for c in cs:
                    sl = slice(c * 512, (c + 1) * 512)
                    for wa, wb, dst in ((wqa, wqb, qro), (wka, wkb, kro)):
                        pa = projp.tile([128, 512], f32, tag="projA", name="pa")
                        pb = projp.tile([128, 512], f32, tag="projB", name="pb")
                        for w, p in ((wa, pa), (wb, pb)):
                            nc.tensor.matmul(
                                p[0:64, :], w[0:64, :], qT[0:64, sl],
                                start=True, stop=True, tile_position=(0, 0),
                            )
                            nc.tensor.matmul(
                                p[64:128, :], w[64:128, :], qT[64:128, sl],
                                start=True, stop=True, tile_position=(64, 64),
                            )
                        tmp1 = big.tile([128, 512], bf16, tag="ropetmp1", name="t1")
                        tmp2 = big.tile([128, 512], bf16, tag="ropetmp2", name="t2")
                        nc.vector.tensor_mul(tmp1[:], pa[:], cosb[:, sl])
                        nc.vector.tensor_mul(tmp2[:], pb[:], sinb[:, sl])
                        nc.vector.tensor_add(dst[:, sl], tmp1[:], tmp2[:])

            def v_strips(vpool, j0):
                # w2 = q @ W_vo tiles for strips j0..j0+7 (64x128 tiles T0/T8)
                vps = [
                    vpool.tile([128, 512], f32, tag="vp0", name="vp0"),
                    vpool.tile([128, 512], f32, tag="vp1", name="vp1"),
                ]
                for j in range(j0, j0 + 8):
                    js = slice(j * 128, (j + 1) * 128)
                    ds = slice((j - j0) * 64, (j - j0 + 1) * 64)
                    nc.tensor.matmul(
                        vps[0][:, ds], qT[0:64, js], wv[0:64, :],
                        start=True, stop=True, tile_position=(0, 0),
                    )
                    nc.tensor.matmul(
                        vps[1][:, ds], qT[64:128, js], wv[64:128, :],
                        start=True, stop=True, tile_position=(64, 0),
                    )
                sb = slice(j0 * 64, (j0 + 8) * 64)
                nc.vector.tensor_copy(vsb[0][:, sb], vps[0][:])
                nc.vector.tensor_copy(vsb[1][:, sb], vps[1][:])

            def sigma(sig):
                # attention for s-half sig, interleaved at 512-col chunk
                # granularity: qk+exp for the strips first needed by this
                # chunk, then sv for the chunk (needs strips j <= 4c+3).
                # pout_c layout: b0 -> [0:64, 0:512] (bank A),
                # b1 -> [64:128, 512:1024] (bank B): batch-disjoint banks so
                # accumulation groups never share a psum zero region.
                s_lo, s_hi = sig * 1024, (sig + 1) * 1024
                with (
                    tc.tile_pool(name=f"qkp{sig}", bufs=2, space="PSUM") as qkp,
                    tc.tile_pool(name=f"op{sig}", bufs=2, space="PSUM") as opool,
                ):
                    for ci in (0, 1):
                        c = 2 * sig + ci
                        lo, hi = c * 512, (c + 1) * 512
                        # strips whose first contribution to this s-half
                        # lands in chunk c
                        strips = [
                            j for j in range(4 * c + 4)
                            if max(2 * sig, j // 4) == c
                        ]
                        for j in strips:
                            base = j * 128
                            a0 = max(s_lo, base)
                            w = s_hi - a0
                            for b in (0, 1):
                                rows = slice(0, 64) if b == 0 else slice(64, 128)
                                tp = (0, 0) if b == 0 else (64, 0)
                                qk = qkp.tile([128, 1024], f32, tag="qk", name="qk")
                                for off in range(0, w, 512):
                                    wc = min(512, w - off)
                                    ssl = slice(a0 + off, a0 + off + wc)
                                    nc.tensor.matmul(
                                        qk[:, off : off + wc],
                                        kro[rows, base : base + 128],
                                        qro[rows, ssl],
                                        start=True, stop=True, tile_position=tp,
                                    )
                                nc.scalar.activation(
                                    scs[(j, b)][:, a0 - base : a0 - base + w],
                                    qk[:, 0:w],
                                    Exp,
                                    bias=kn[b][:, j : j + 1],
                                    scale=1.0,
                                )
                            if a0 == base:
                                # diagonal block of strip j is in this half
                                for b in (0, 1):
                                    nc.vector.tensor_mul(
                                        scs[(j, b)][:, 0:128],
                                        scs[(j, b)][:, 0:128],
                                        mask[:],
                                    )
                        # sv for chunk c: all strips j <= 4c+3 are ready
                        pout = opool.tile([128, 1024], f32, tag="pout", name="pout")
                        nj = 4 * (c + 1)
                        for j in range(nj):
                            a = max(lo, j * 128)
                            for b in (0, 1):
                                orows = slice(0, 64) if b == 0 else slice(64, 128)
                                tp = (0, 0) if b == 0 else (0, 64)
                                po = 512 * b + a - lo
                                nc.tensor.matmul(
                                    pout[orows, po : po + hi - a],
                                    vsb[b][:, j * 64 : (j + 1) * 64],
                                    scs[(j, b)][:, a - j * 128 : hi - j * 128],
                                    start=(j == 0),
                                    stop=(j == nj - 1),
                                    tile_position=tp,
                                )
                        nc.vector.tensor_copy(
                            outsb[0:64, lo:hi], pout[0:64, 0:512]
                        )
                        nc.vector.tensor_copy(
                            outsb[64:128, lo:hi], pout[64:128, 512:1024]
                        )
                        nc.sync.dma_start(t_out[:, lo:hi], outsb[:, lo:hi])

            with tc.tile_pool(name="projp1", bufs=2, space="PSUM") as projp:
                proj_chunks(projp, (0, 1))
            with tc.tile_pool(name="vp1", bufs=1, space="PSUM") as vpool:
                v_strips(vpool, 0)
            sigma(0)
            with tc.tile_pool(name="projp2", bufs=2, space="PSUM") as projp:
                proj_chunks(projp, (2, 3))
            with tc.tile_pool(name="vp2", bufs=1, space="PSUM") as vpool:
                v_strips(vpool, 8)
            sigma(1)

    nc.compile()
    return nc


def _get_program():
    global _PROG
    if _PROG is None:
        _PROG = _build_program()
    return _PROG


def _prep_inputs(q, Wq, Wk, Wv, Wo, gamma):
    """Build the per-core in_maps (all host-side numpy)."""
    q = np.asarray(q, np.float32)
    Wq = np.asarray(Wq, np.float32)
    Wk = np.asarray(Wk, np.float32)
    Wv = np.asarray(Wv, np.float32)
    Wo = np.asarray(Wo, np.float32)
    gamma = np.asarray(gamma, np.float32)

    perm = np.concatenate([np.arange(0, 64, 2), np.arange(1, 64, 2)])
    f = np.arange(32, dtype=np.float64)
    freqs = 1.0 / (10000.0 ** (2 * f / 64))
    ang = np.arange(S, dtype=np.float64)[:, None] * freqs[None, :]
    cosr = np.cos(ang).T.astype(np.float32)  # [32, S]
    sinr = np.sin(ang).T.astype(np.float32)
    C64 = np.concatenate([cosr, cosr], 0)  # [64, S]
    S64 = np.concatenate([sinr, sinr], 0)
    cosb = np.concatenate([C64, C64], 0).astype(BF16)  # [128, S]
    sinb = np.concatenate([S64, S64], 0).astype(BF16)

    qT_packed = np.concatenate([q[0].T, q[1].T], 0).astype(BF16)  # [128, S]
    mask = np.triu(np.ones((128, 128), np.float32)).astype(BF16)

    def dup(x):
        return np.concatenate([x, x], 0)

    in_maps = []
    qn_exp = np.zeros((B, H, S), np.float32)
    for h in range(H):
        g = float(gamma[h]) * SCALE
        Wq_h = Wq[h * 64 : (h + 1) * 64]
        Wk_h = Wk[h * 64 : (h + 1) * 64]
        Wv_h = Wv[h * 64 : (h + 1) * 64]
        Wo_h = Wo[:, h * 64 : (h + 1) * 64]  # [64(e), 64(d)]
        W_vo = Wv_h.T @ Wo_h.T  # [64(i), 64(e)] : q @ W_vo = vh @ Wo_h.T
        A_q = Wq_h[perm]
        B_q = np.concatenate([-Wq_h[1::2], Wq_h[0::2]], 0)
        A_k = Wk_h[perm] * (2.0 * g)
        B_k = np.concatenate([-Wk_h[1::2], Wk_h[0::2]], 0) * (2.0 * g)

        knb = []
        for b in range(B):
            kh = q[b] @ Wk_h.T
            kn = (kh * kh).sum(-1)  # [S]
            knb.append((-g * kn).reshape(16, 128).T.astype(np.float32))
            qh = q[b] @ Wq_h.T
            qn = (qh * qh).sum(-1)
            qn_exp[b, h] = np.exp(-g * qn)

        wcat = np.concatenate(
            [
                dup(A_q.T).astype(BF16),
                dup(B_q.T).astype(BF16),
                dup(A_k.T).astype(BF16),
                dup(B_k.T).astype(BF16),
                dup(W_vo).astype(BF16),
                mask,
            ],
            axis=1,
        )
        kncat = np.ascontiguousarray(
            np.concatenate([knb[0], knb[1]], axis=1), dtype=np.float32
        )
        in_maps.append(
            {
                "qT": qT_packed,
                "wcat": np.ascontiguousarray(wcat),
                "cosb": cosb,
                "sinb": sinb,
                "kncat": kncat,
            }
        )
    return in_maps, qn_exp


def kernel(q, Wq, Wk, Wv, Wo, gamma):
    global LAST_RESULTS
    from concourse import bass_utils

    nc = _get_program()
    in_maps, qn_exp = _prep_inputs(q, Wq, Wk, Wv, Wo, gamma)
    trace = bool(int(os.environ.get("KERNEL_TRACE", "0")))
    res = bass_utils.run_bass_kernel_spmd(
        nc, in_maps, core_ids=list(range(N_CORES)), trace=trace
    )
    LAST_RESULTS = res

    final = np.zeros((B, S, D), np.float32)
    for h in range(H):
        o = np.asarray(res.results[h]["out"], np.float32)  # [128, S]
        for b in range(B):
            final[b] += (o[b * 64 : (b + 1) * 64, :] * qn_exp[b, h][None, :]).T
    return final


# revision 17
# speedup vs baseline: 1.3135x; 1.1793x over previous
"""Trainium2 Bass kernel for nn_Attention_15771119911478 (RBF attention w/ RoPE).

Sharding: core h (of 8) computes head h for both batches. Per-core output is
the head's contribution to out @ Wo.T, transposed ([2*64(e), 2048(s)]), minus
a per-row factor exp(-g*qn[s]) which is applied on the host (it commutes
through the Wo projection). Host sums the 8 per-core partials.

Device math per core (batch b packed on partition halves):
  qro = (A_q q^T) * C + (B_q q^T) * S          (RoPE as two projections)
  kro = 2g * [(A_k q^T) * C + (B_k q^T) * S]   (2g folded into Wk)
  scT[t,s] = exp(kro[:,t].qro[:,s] - g*kn[t])  (kn bias folded into ACT exp)
  out_hT = vh^T @ (scT masked t<=s)
  partial_T = Wo_h @ out_hT                     -> DMA out [128, 2048] f32
"""
import os
import sys

sys.path.insert(0, "/opt/trn_rl_repo")

import numpy as np
import ml_dtypes

S = 2048
D = 64
H = 8
B = 2
N_CORES = 8
SCALE = 1.0 / 8.0  # 1/sqrt(64)
BF16 = ml_dtypes.bfloat16

_PROG = None  # cached (nc, tensors) after first build
LAST_RESULTS = None  # BassKernelResults of last run (for test.py)


def _build_program():
    import concourse.bass as bass
    import concourse.bacc as bacc
    import concourse.tile as tile
    from concourse import mybir

    f32 = mybir.dt.float32
    bf16 = mybir.dt.bfloat16
    Exp = mybir.ActivationFunctionType.Exp

    nc = bacc.Bacc(
        "TRN2",
        target_bir_lowering=False,
        debug=False,
        enable_asserts=False,
        num_devices=N_CORES,
    )

    def din(name, shape, dt):
        return nc.dram_tensor(name, shape, dt, kind="ExternalInput").ap()

    t_qT = din("qT", [128, S], bf16)
    t_wcat = din("wcat", [128, 448], bf16)  # wqa|wqb|wka|wkb|wv|mask
    t_cos = din("cosb", [128, S], bf16)
    t_sin = din("sinb", [128, S], bf16)
    t_kncat = din("kncat", [128, 32], f32)  # knb0|knb1
    t_out = nc.dram_tensor("out", [128, S], f32, kind="ExternalOutput").ap()

    with tile.TileContext(nc) as tc:
        with (
            tc.tile_pool(name="const", bufs=1) as const,
            tc.tile_pool(name="big", bufs=1) as big,
            tc.tile_pool(name="scp", bufs=1) as scp,
        ):
            # ---- load inputs to SBUF ----
            qT = big.tile([128, S], bf16, tag="qT")
            cosb = big.tile([128, S], bf16, tag="cosb")
            sinb = big.tile([128, S], bf16, tag="sinb")
            wcat = const.tile([128, 448], bf16, tag="wcat")
            kncat = const.tile([128, 32], f32, tag="kncat")
            nc.sync.dma_start(wcat[:], t_wcat[:])
            wqa, wqb = wcat[:, 0:64], wcat[:, 64:128]
            wka, wkb = wcat[:, 128:192], wcat[:, 192:256]
            wv = wcat[:, 256:320]
            mask = wcat[:, 320:448]
            # chunked loads, first chunk right after the consts so the
            # proj/rope/attention pipeline starts immediately
            for c in range(4):
                sl = slice(c * 512, (c + 1) * 512)
                nc.sync.dma_start(qT[:, sl], t_qT[:, sl])
                nc.sync.dma_start(cosb[:, sl], t_cos[:, sl])
                nc.sync.dma_start(sinb[:, sl], t_sin[:, sl])
                if c == 0:
                    nc.sync.dma_start(kncat[:], t_kncat[:])
            kn = [kncat[:, 0:16], kncat[:, 16:32]]

            # preload ACT exp table early (overlaps with projections)
            scratch = const.tile([128, 1], f32, tag="scratch")
            nc.scalar.activation(scratch[:], kncat[:, 0:1], Exp)

            # ---- q/k projections + RoPE (64x64 tiles T0/T10) ----
            qro = big.tile([128, S], bf16, tag="qro")
            kro = big.tile([128, S], bf16, tag="kro")
            vsb = [
                big.tile([128, 1024], bf16, tag="vsb0", name="vsb0"),
                big.tile([128, 1024], bf16, tag="vsb1", name="vsb1"),
            ]
            # sc strips: scs[(j,b)] covers scores^T rows t in [128j,128j+128),
            # cols s in [128j, 2048)
            scs = {}
            for j in range(16):
                cols = S - j * 128
                for b in (0, 1):
                    scs[(j, b)] = scp.tile(
                        [128, cols], bf16, tag=f"sc_{j}_{b}", name=f"sc_{j}_{b}"
                    )
            outsb = big.tile([128, S], f32, tag="outsb")

            def proj_chunks(projp, cs, which="qk"):
                for c in cs:
                    sl = slice(c * 512, (c + 1) * 512)
                    sel = []
                    if "q" in which:
                        sel.append((wqa, wqb, qro))
                    if "k" in which:
                        sel.append((wka, wkb, kro))
                    for wa, wb, dst in sel:
                        pa = projp.tile([128, 512], f32, tag="projA", name="pa")
                        pb = projp.tile([128, 512], f32, tag="projB", name="pb")
                        for w, p in ((wa, pa), (wb, pb)):
                            nc.tensor.matmul(
                                p[0:64, :], w[0:64, :], qT[0:64, sl],
                                start=True, stop=True, tile_position=(0, 0),
                            )
                            nc.tensor.matmul(
                                p[64:128, :], w[64:128, :], qT[64:128, sl],
                                start=True, stop=True, tile_position=(64, 64),
                            )
                        tmp1 = big.tile([128, 512], bf16, tag="ropetmp1", name="t1")
                        tmp2 = big.tile([128, 512], bf16, tag="ropetmp2", name="t2")
                        nc.vector.tensor_mul(tmp1[:], pa[:], cosb[:, sl])
                        nc.vector.tensor_mul(tmp2[:], pb[:], sinb[:, sl])
                        nc.vector.tensor_add(dst[:, sl], tmp1[:], tmp2[:])

            def v_strips(vpool, j0):
                # w2 = q @ W_vo tiles for strips j0..j0+7 (64x128 tiles T0/T8)
                vps = [
                    vpool.tile([128, 512], f32, tag="vp0", name="vp0"),
                    vpool.tile([128, 512], f32, tag="vp1", name="vp1"),
                ]
                for j in range(j0, j0 + 8):
                    js = slice(j * 128, (j + 1) * 128)
                    ds = slice((j - j0) * 64, (j - j0 + 1) * 64)
                    nc.tensor.matmul(
                        vps[0][:, ds], qT[0:64, js], wv[0:64, :],
                        start=True, stop=True, tile_position=(0, 0),
                    )
                    nc.tensor.matmul(
                        vps[1][:, ds], qT[64:128, js], wv[64:128, :],
                        start=True, stop=True, tile_position=(64, 0),
                    )
                sb = slice(j0 * 64, (j0 + 8) * 64)
                nc.vector.tensor_copy(vsb[0][:, sb], vps[0][:])
                nc.vector.tensor_copy(vsb[1][:, sb], vps[1][:])

            def sigma(sig, pre_sv0=None, mid_work=None):
                # attention for s-half sig, interleaved at 512-col chunk
                # granularity. sv matmuls for chunk c are emitted lagging one
                # strip behind the qk/exp stream, so they fill PE while ACT
                # exps later strips instead of trailing after the last exp.
                # pout_c layout: b0 -> [0:64, 0:512] (bank A),
                # b1 -> [64:128, 512:1024] (bank B): batch-disjoint banks so
                # accumulation groups never share a psum zero region.
                s_lo, s_hi = sig * 1024, (sig + 1) * 1024
                with (
                    tc.tile_pool(name=f"qkp{sig}", bufs=2, space="PSUM") as qkp,
                    tc.tile_pool(name=f"op{sig}", bufs=1, space="PSUM") as opool,
                ):
                    for ci in (0, 1):
                        c = 2 * sig + ci
                        lo, hi = c * 512, (c + 1) * 512
                        nj = 4 * (c + 1)
                        strips = [
                            j for j in range(nj)
                            if max(2 * sig, j // 4) == c
                        ]
                        for si, j in enumerate(strips):
                            base = j * 128
                            a0 = max(s_lo, base)
                            w = s_hi - a0
                            for b in (0, 1):
                                rows = slice(0, 64) if b == 0 else slice(64, 128)
                                tp = (0, 0) if b == 0 else (64, 0)
                                qk = qkp.tile([128, 1024], f32, tag="qk", name="qk")
                                for off in range(0, w, 512):
                                    wc = min(512, w - off)
                                    ssl = slice(a0 + off, a0 + off + wc)
                                    nc.tensor.matmul(
                                        qk[:, off : off + wc],
                                        kro[rows, base : base + 128],
                                        qro[rows, ssl],
                                        start=True, stop=True, tile_position=tp,
                                    )
                                nc.scalar.activation(
                                    scs[(j, b)][:, a0 - base : a0 - base + w],
                                    qk[:, 0:w],
                                    Exp,
                                    bias=kn[b][:, j : j + 1],
                                    scale=1.0,
                                )
                            if a0 == base:
                                # diagonal block of strip j is in this half
                                for b in (0, 1):
                                    nc.vector.tensor_mul(
                                        scs[(j, b)][:, 0:128],
                                        scs[(j, b)][:, 0:128],
                                        mask[:],
                                    )
                            if si == 0 and ci == 0 and pre_sv0 is not None:
                                pre_sv0()
                        # sv for chunk c: all strips j <= 4c+3 are ready
                        pout = opool.tile([128, 1024], f32, tag="pout", name="pout")
                        for jj in range(nj):
                            a = max(lo, jj * 128)
                            for b in (0, 1):
                                orows = slice(0, 64) if b == 0 else slice(64, 128)
                                tp = (0, 0) if b == 0 else (0, 64)
                                po = 512 * b + a - lo
                                nc.tensor.matmul(
                                    pout[orows, po : po + hi - a],
                                    vsb[b][:, jj * 64 : (jj + 1) * 64],
                                    scs[(jj, b)][:, a - jj * 128 : hi - jj * 128],
                                    start=(jj == 0),
                                    stop=(jj == nj - 1),
                                    tile_position=tp,
                                )
                        nc.vector.tensor_copy(
                            outsb[0:64, lo:hi], pout[0:64, 0:512]
                        )
                        nc.vector.tensor_copy(
                            outsb[64:128, lo:hi], pout[64:128, 512:1024]
                        )
                        nc.sync.dma_start(t_out[:, lo:hi], outsb[:, lo:hi])
                        if ci == 0 and mid_work is not None:
                            mid_work()

            def v_early():
                # runs after sigma0's first strip: psum = qkp(4) -> 4 free.
                # k-proj of chunk 1 was deferred here (first needed by strip
                # j4); then the first 8 w2 strips for sv c0.
                with tc.tile_pool(name="pk1", bufs=1, space="PSUM") as pk:
                    proj_chunks(pk, (1,), which="k")
                with tc.tile_pool(name="vp1", bufs=1, space="PSUM") as vpool:
                    v_strips(vpool, 0)

            def boundary_work():
                # runs inside sigma0 after chunk c0 is evacuated:
                # psum = qkp(4) + op(2) -> 2 free
                with tc.tile_pool(name="projp2", bufs=1, space="PSUM") as pp:
                    proj_chunks(pp, (2, 3))
                with tc.tile_pool(name="vp2", bufs=1, space="PSUM") as vpool:
                    v_strips(vpool, 8)

            with tc.tile_pool(name="projp1", bufs=2, space="PSUM") as projp:
                proj_chunks(projp, (0,))
                proj_chunks(projp, (1,), which="q")
            sigma(0, pre_sv0=v_early, mid_work=boundary_work)
            sigma(1)

    nc.compile()
    return nc


def _get_program():
    global _PROG
    if _PROG is None:
        _PROG = _build_program()
    return _PROG


def _prep_inputs(q, Wq, Wk, Wv, Wo, gamma):
    """Build the per-core in_maps (all host-side numpy)."""
    q = np.asarray(q, np.float32)
    Wq = np.asarray(Wq, np.float32)
    Wk = np.asarray(Wk, np.float32)
    Wv = np.asarray(Wv, np.float32)
    Wo = np.asarray(Wo, np.float32)
    gamma = np.asarray(gamma, np.float32)

    perm = np.concatenate([np.arange(0, 64, 2), np.arange(1, 64, 2)])
    f = np.arange(32, dtype=np.float64)
    freqs = 1.0 / (10000.0 ** (2 * f / 64))
    ang = np.arange(S, dtype=np.float64)[:, None] * freqs[None, :]
    cosr = np.cos(ang).T.astype(np.float32)  # [32, S]
    sinr = np.sin(ang).T.astype(np.float32)
    C64 = np.concatenate([cosr, cosr], 0)  # [64, S]
    S64 = np.concatenate([sinr, sinr], 0)
    cosb = np.concatenate([C64, C64], 0).astype(BF16)  # [128, S]
    sinb = np.concatenate([S64, S64], 0).astype(BF16)

    qT_packed = np.concatenate([q[0].T, q[1].T], 0).astype(BF16)  # [128, S]
    mask = np.triu(np.ones((128, 128), np.float32)).astype(BF16)

    def dup(x):
        return np.concatenate([x, x], 0)

    in_maps = []
    qn_exp = np.zeros((B, H, S), np.float32)
    for h in range(H):
        g = float(gamma[h]) * SCALE
        Wq_h = Wq[h * 64 : (h + 1) * 64]
        Wk_h = Wk[h * 64 : (h + 1) * 64]
        Wv_h = Wv[h * 64 : (h + 1) * 64]
        Wo_h = Wo[:, h * 64 : (h + 1) * 64]  # [64(e), 64(d)]
        W_vo = Wv_h.T @ Wo_h.T  # [64(i), 64(e)] : q @ W_vo = vh @ Wo_h.T
        A_q = Wq_h[perm]
        B_q = np.concatenate([-Wq_h[1::2], Wq_h[0::2]], 0)
        A_k = Wk_h[perm] * (2.0 * g)
        B_k = np.concatenate([-Wk_h[1::2], Wk_h[0::2]], 0) * (2.0 * g)

        knb = []
        for b in range(B):
            kh = q[b] @ Wk_h.T
            kn = (kh * kh).sum(-1)  # [S]
            knb.append((-g * kn).reshape(16, 128).T.astype(np.float32))
            qh = q[b] @ Wq_h.T
            qn = (qh * qh).sum(-1)
            qn_exp[b, h] = np.exp(-g * qn)

        wcat = np.concatenate(
            [
                dup(A_q.T).astype(BF16),
                dup(B_q.T).astype(BF16),
                dup(A_k.T).astype(BF16),
                dup(B_k.T).astype(BF16),
                dup(W_vo).astype(BF16),
                mask,
            ],
            axis=1,
        )
        kncat = np.ascontiguousarray(
            np.concatenate([knb[0], knb[1]], axis=1), dtype=np.float32
        )
        in_maps.append(
            {
                "qT": qT_packed,
                "wcat": np.ascontiguousarray(wcat),
                "cosb": cosb,
                "sinb": sinb,
                "kncat": kncat,
            }
        )
    return in_maps, qn_exp


def kernel(q, Wq, Wk, Wv, Wo, gamma):
    global LAST_RESULTS
    from concourse import bass_utils

    nc = _get_program()
    in_maps, qn_exp = _prep_inputs(q, Wq, Wk, Wv, Wo, gamma)
    trace = bool(int(os.environ.get("KERNEL_TRACE", "0")))
    res = bass_utils.run_bass_kernel_spmd(
        nc, in_maps, core_ids=list(range(N_CORES)), trace=trace
    )
    LAST_RESULTS = res

    final = np.zeros((B, S, D), np.float32)
    for h in range(H):
        o = np.asarray(res.results[h]["out"], np.float32)  # [128, S]
        for b in range(B):
            final[b] += (o[b * 64 : (b + 1) * 64, :] * qn_exp[b, h][None, :]).T
    return final
